# revision 1
# baseline (speedup 1.0000x reference)
"""GroupingBlock kernel for 8 Trainium2 NeuronCores.

Strategy: data-parallel over batch B=32 -> 4 batches per core, all params
replicated (matches the sharding hint). The whole block is compiled once
with XLA-Neuron and executed on all 8 cores via jax.pmap; inputs are
sharded/replicated on the host, outputs gathered back to a full
[32, 64, 768] fp32 array.

Hardcoded problem shapes (self-contained; reads nothing from disk):
  x:[32,4096,768] group_tokens:[32,128,768] C=768 H=12 G_OUT=64
"""

import numpy as np

B, N, G_IN, G_OUT, C, H = 32, 4096, 128, 64, 768, 12
NCORES = 8
BL = B // NCORES  # 4 batches per core

_WEIGHT_NAMES = [
    'ln_tokens_g', 'ln_tokens_b', 'ln_x_g', 'ln_x_b',
    'inter_w1', 'inter_b1', 'inter_w2', 'inter_b2', 'ln_pt_g', 'ln_pt_b',
    'ca_qw', 'ca_qb', 'ca_kw', 'ca_kb', 'ca_vw', 'ca_vb', 'ca_pw', 'ca_pb',
    'ca_ln2_g', 'ca_ln2_b', 'ca_m1w', 'ca_m1b', 'ca_m2w', 'ca_m2b',
    'ca_lnp_g', 'ca_lnp_b',
    'as_qw', 'as_qb', 'as_kw', 'as_kb', 'as_vw', 'as_vb', 'as_pw', 'as_pb',
    'ln_nx_g', 'ln_nx_b', 'mc_w1', 'mc_b1', 'mc_w2', 'mc_b2',
]

_pmapped = None


def _build():
    global _pmapped
    if _pmapped is not None:
        return _pmapped
    import jax
    import jax.numpy as jnp

    def _ln(x, g, b, eps=1e-5):
        m = jnp.mean(x, -1, keepdims=True)
        v = jnp.mean((x - m) ** 2, -1, keepdims=True)
        return (x - m) * jax.lax.rsqrt(v + eps) * g + b

    def _heads(t, h):
        b, s, c = t.shape
        return t.reshape(b, s, h, c // h).transpose(0, 2, 1, 3)

    def block(x, group_tokens, w):
        gt = _ln(group_tokens, w['ln_tokens_g'], w['ln_tokens_b'])
        xn = _ln(x, w['ln_x_g'], w['ln_x_b'])
        t = gt.transpose(0, 2, 1)
        t = jax.nn.gelu(t @ w['inter_w1'] + w['inter_b1'], approximate=False)
        t = t @ w['inter_w2'] + w['inter_b2']
        pgt = _ln(t.transpose(0, 2, 1), w['ln_pt_g'], w['ln_pt_b'])
        scale = (C // H) ** -0.5
        q = _heads(pgt @ w['ca_qw'] + w['ca_qb'], H)
        k = _heads(xn @ w['ca_kw'] + w['ca_kb'], H)
        v = _heads(xn @ w['ca_vw'] + w['ca_vb'], H)
        attn = jax.nn.softmax(jnp.einsum('bhgc,bhnc->bhgn', q, k) * scale, -1)
        o = jnp.einsum('bhgn,bhnc->bhgc', attn, v)
        o = o.transpose(0, 2, 1, 3).reshape(-1, G_OUT, C)
        y = pgt + (o @ w['ca_pw'] + w['ca_pb'])
        y = y + (jax.nn.gelu(_ln(y, w['ca_ln2_g'], w['ca_ln2_b']) @ w['ca_m1w']
                             + w['ca_m1b'], approximate=False)
                 @ w['ca_m2w'] + w['ca_m2b'])
        pgt2 = _ln(y, w['ca_lnp_g'], w['ca_lnp_b'])
        # AssignAttention (eval): straight-through == hard one-hot assignment.
        aq = pgt2 @ w['as_qw'] + w['as_qb']            # [b,G,C]
        ak = xn @ w['as_kw'] + w['as_kb']              # [b,N,C]
        av = xn @ w['as_vw'] + w['as_vb']              # [b,N,C]
        raw = jnp.einsum('bgc,bnc->bgn', aq, ak)       # argmax invariant to *scale
        idx = jnp.argmax(raw, axis=-2)                 # [b,N]
        a = jax.nn.one_hot(idx, G_OUT, axis=-2, dtype=raw.dtype)  # [b,G,N]
        a = a / (jnp.sum(a, -1, keepdims=True) + 1.0)
        new_x = jnp.einsum('bgn,bnc->bgc', a, av)
        new_x = new_x @ w['as_pw'] + w['as_pb'] + pgt2
        out = new_x + (jax.nn.gelu(_ln(new_x, w['ln_nx_g'], w['ln_nx_b'])
                                   @ w['mc_w1'] + w['mc_b1'], approximate=False)
                       @ w['mc_w2'] + w['mc_b2'])
        return out

    _pmapped = jax.pmap(block, in_axes=(0, 0, None), devices=jax.devices()[:NCORES])
    return _pmapped


def kernel(**inputs):
    fn = _build()
    x = np.ascontiguousarray(inputs['x'], dtype=np.float32)
    gt = np.ascontiguousarray(inputs['group_tokens'], dtype=np.float32)
    xs = x.reshape(NCORES, BL, N, C)
    gts = gt.reshape(NCORES, BL, G_IN, C)
    w = {k: np.asarray(inputs[k], dtype=np.float32) for k in _WEIGHT_NAMES}
    out = fn(xs, gts, w)
    return np.asarray(out).reshape(B, G_OUT, C).astype(np.float32)


# revision 3
# speedup vs baseline: 247.6664x; 247.6664x over previous
"""GroupingBlock kernel for 8 Trainium2 NeuronCores.

Strategy: data-parallel over batch B=32 -> 4 batches per core, all params
replicated (matches the sharding hint). The whole block is compiled once
with XLA-Neuron and executed on all 8 cores via jax.pmap; inputs are
sharded/replicated on the host, outputs gathered back to a full
[32, 64, 768] fp32 array.

Hardcoded problem shapes (self-contained; reads nothing from disk):
  x:[32,4096,768] group_tokens:[32,128,768] C=768 H=12 G_OUT=64
"""

import numpy as np

B, N, G_IN, G_OUT, C, H = 32, 4096, 128, 64, 768, 12
NCORES = 8
BL = B // NCORES  # 4 batches per core

_WEIGHT_NAMES = [
    'ln_tokens_g', 'ln_tokens_b', 'ln_x_g', 'ln_x_b',
    'inter_w1', 'inter_b1', 'inter_w2', 'inter_b2', 'ln_pt_g', 'ln_pt_b',
    'ca_qw', 'ca_qb', 'ca_kw', 'ca_kb', 'ca_vw', 'ca_vb', 'ca_pw', 'ca_pb',
    'ca_ln2_g', 'ca_ln2_b', 'ca_m1w', 'ca_m1b', 'ca_m2w', 'ca_m2b',
    'ca_lnp_g', 'ca_lnp_b',
    'as_qw', 'as_qb', 'as_kw', 'as_kb', 'as_vw', 'as_vb', 'as_pw', 'as_pb',
    'ln_nx_g', 'ln_nx_b', 'mc_w1', 'mc_b1', 'mc_w2', 'mc_b2',
]

_pmapped = None
_block_fn = None


def _build():
    global _pmapped, _block_fn
    if _pmapped is not None:
        return _pmapped
    import jax
    import jax.numpy as jnp

    def _ln(x, g, b, eps=1e-5):
        m = jnp.mean(x, -1, keepdims=True)
        v = jnp.mean((x - m) ** 2, -1, keepdims=True)
        return (x - m) * jax.lax.rsqrt(v + eps) * g + b

    def _heads(t, h):
        b, s, c = t.shape
        return t.reshape(b, s, h, c // h).transpose(0, 2, 1, 3)

    def block(x, group_tokens, w):
        gt = _ln(group_tokens, w['ln_tokens_g'], w['ln_tokens_b'])
        xn = _ln(x, w['ln_x_g'], w['ln_x_b'])
        t = gt.transpose(0, 2, 1)
        t = jax.nn.gelu(t @ w['inter_w1'] + w['inter_b1'], approximate=False)
        t = t @ w['inter_w2'] + w['inter_b2']
        pgt = _ln(t.transpose(0, 2, 1), w['ln_pt_g'], w['ln_pt_b'])
        scale = (C // H) ** -0.5
        q = _heads(pgt @ w['ca_qw'] + w['ca_qb'], H)
        k = _heads(xn @ w['ca_kw'] + w['ca_kb'], H)
        v = _heads(xn @ w['ca_vw'] + w['ca_vb'], H)
        attn = jax.nn.softmax(jnp.einsum('bhgc,bhnc->bhgn', q, k) * scale, -1)
        o = jnp.einsum('bhgn,bhnc->bhgc', attn, v)
        o = o.transpose(0, 2, 1, 3).reshape(-1, G_OUT, C)
        y = pgt + (o @ w['ca_pw'] + w['ca_pb'])
        y = y + (jax.nn.gelu(_ln(y, w['ca_ln2_g'], w['ca_ln2_b']) @ w['ca_m1w']
                             + w['ca_m1b'], approximate=False)
                 @ w['ca_m2w'] + w['ca_m2b'])
        pgt2 = _ln(y, w['ca_lnp_g'], w['ca_lnp_b'])
        # AssignAttention (eval): straight-through == hard one-hot assignment.
        aq = pgt2 @ w['as_qw'] + w['as_qb']            # [b,G,C]
        ak = xn @ w['as_kw'] + w['as_kb']              # [b,N,C]
        av = xn @ w['as_vw'] + w['as_vb']              # [b,N,C]
        raw = jnp.einsum('bgc,bnc->bgn', aq, ak)       # argmax invariant to *scale
        idx = jnp.argmax(raw, axis=-2)                 # [b,N]
        a = jax.nn.one_hot(idx, G_OUT, axis=-2, dtype=raw.dtype)  # [b,G,N]
        a = a / (jnp.sum(a, -1, keepdims=True) + 1.0)
        new_x = jnp.einsum('bgn,bnc->bgc', a, av)
        new_x = new_x @ w['as_pw'] + w['as_pb'] + pgt2
        out = new_x + (jax.nn.gelu(_ln(new_x, w['ln_nx_g'], w['ln_nx_b'])
                                   @ w['mc_w1'] + w['mc_b1'], approximate=False)
                       @ w['mc_w2'] + w['mc_b2'])
        return out

    _block_fn = block
    _pmapped = jax.pmap(block, in_axes=(0, 0, None), devices=jax.devices()[:NCORES])
    return _pmapped


def kernel(**inputs):
    fn = _build()
    x = np.ascontiguousarray(inputs['x'], dtype=np.float32)
    gt = np.ascontiguousarray(inputs['group_tokens'], dtype=np.float32)
    xs = x.reshape(NCORES, BL, N, C)
    gts = gt.reshape(NCORES, BL, G_IN, C)
    w = {k: np.asarray(inputs[k], dtype=np.float32) for k in _WEIGHT_NAMES}
    out = fn(xs, gts, w)
    return np.asarray(out).reshape(B, G_OUT, C).astype(np.float32)
